# revision 1
# baseline (speedup 1.0000x reference)
"""Trainium2 Bass kernel for nn_MultiHeadAttention_86457691669080.

Sharding: (batch, head-group) over 8 cores — core c handles batch c//2 and
heads (c%2)*8..(c%2)*8+8.  Each core runs the full pipeline for its shard in
"transposed" layout (feature dim on partitions, sequence on the free dim):

  P1: Q^T/K^T projections ([dq, n]), V in natural layout with a fused ones
      column (V' = [V | 1]) so the AV matmul also emits softmax denominators.
  P2: per (head, q-half): S^T = K_h Q_h^T (PE, K=64), X = exp(S/sqrt(dk))
      (ACT), E = X*exp(adj)^T and Em = X*(exp(adj)*mask)^T (DVE/GPSIMD —
      exp(adj) is precomputed on the host, which both removes the adj add
      from the scores and keeps the on-device exponent small), then
      G|L = V'^T E / V'^T Em accumulated in PSUM; row 64 = softmax sums.
      R = exp(-ln(s)) (ACT), broadcast across partitions via a DRAM
      round-trip DMA, then tmp = rho*(G*Rg) + (L*Rl) on DVE.  The sigmoid
      gate folds into rho = a/(1-a): the global (1-a) factor cancels in the
      downstream L2 normalization.
  P3: signed-sqrt (|x| via abs_max, ACT sqrt, sign bit OR'd back) and L2
      normalization over the sequence axis (free-dim accum of |tmp| = sum of
      signed-sqrt squares).
  P4: output projection against Wo[:, group]^T; host sums the two partial
      products per batch and adds bo.

Matmuls run in bf16 (fp32 accumulation); softmax statistics stay fp32.
"""

import numpy as np
import ml_dtypes

import concourse.bass as bass
import concourse.mybir as mybir
import concourse.tile as tile
from concourse import bacc
from concourse.bass_utils import run_bass_kernel_spmd

AF = mybir.ActivationFunctionType
ALU = mybir.AluOpType
BF16 = mybir.dt.bfloat16
F32 = mybir.dt.float32

B, N, D = 4, 1024, 1024
H = 16
HD = 64
NORM = 1.0 / np.sqrt(1024.0)
HL = 8          # heads per core
DQL = 512       # local projection width (8 heads * 64)
NCORES = 8

_CACHE = {}
TRACE = False  # set by test harness to collect an NTFF profile

# Restrict the activation-table-load pass to two sets that jointly cover
# every ACT function used here (Exp/Ln/Identity/Copy in one, Sqrt in the
# other).  Left to its own devices the pass alternates exp/natural_log sets
# per attention unit — 34 table loads at ~2.7us each.  Indices must be
# preserved (act_func_set_id indexes the full act_info.json list), so
# unwanted sets are emptied rather than removed.
_ACT_SETS_KEEP = {"natural_log_exp_and_others"}
_orig_get_activation_tables = None


def _patched_get_activation_tables(arch):
    t = _orig_get_activation_tables(arch)
    return {k: (v if k in _ACT_SETS_KEEP else set()) for k, v in t.items()}


def _install_act_table_patch():
    global _orig_get_activation_tables
    if _orig_get_activation_tables is None:
        import concourse.bacc as _bacc_mod
        _orig_get_activation_tables = _bacc_mod.get_activation_tables
        _bacc_mod.get_activation_tables = _patched_get_activation_tables


def _build(rho: float, repeat: int = 1):
    _install_act_table_patch()
    nc = bacc.Bacc()
    xq_p = nc.declare_dram_parameter("xq", [D, N], BF16, isOutput=False)
    xk_p = nc.declare_dram_parameter("xk", [D, N], BF16, isOutput=False)
    xv_p = nc.declare_dram_parameter("xv", [D, N], BF16, isOutput=False)
    wq_p = nc.declare_dram_parameter("wq", [D, DQL], BF16, isOutput=False)
    wk_p = nc.declare_dram_parameter("wk", [D, DQL], BF16, isOutput=False)
    wv_p = nc.declare_dram_parameter("wv", [D, DQL], BF16, isOutput=False)
    bq_p = nc.declare_dram_parameter("bq", [128, 4], F32, isOutput=False)
    bk_p = nc.declare_dram_parameter("bk", [128, 4], F32, isOutput=False)
    bv_p = nc.declare_dram_parameter("bv", [1, DQL], F32, isOutput=False)
    at_p = nc.declare_dram_parameter("at", [N, N], BF16, isOutput=False)
    amt_p = nc.declare_dram_parameter("amt", [N, N], BF16, isOutput=False)
    wo_p = nc.declare_dram_parameter("wo", [DQL, D], BF16, isOutput=False)
    out_p = nc.declare_dram_parameter("out", [D, N], F32, isOutput=True)
    r_dram = nc.dram_tensor("r_scratch", [16, N], BF16)

    for _rep in range(repeat):
      with tile.TileContext(nc) as tc:
        with tc.tile_pool(name="singles", bufs=1) as singles:
            # ---- resident SBUF tensors ----
            bq_sb = singles.tile([128, 4], F32)
            bk_sb = singles.tile([128, 4], F32)
            bvb_sb = singles.tile([128, DQL], F32)
            a_sb = singles.tile([128, 8, N], BF16)
            am_sb = singles.tile([128, 8, N], BF16)
            wo_sb = singles.tile([128, 4, N], BF16)
            qt_sb = singles.tile([128, 4, N], BF16)
            kt_sb = singles.tile([128, 4, N], BF16)
            vp_sb = singles.tile([128, 8, HL, 65], BF16)
            tmp_sb = singles.tile([128, 4, N], BF16)
            y_sb = singles.tile([128, 4, N], BF16)
            nrm2_sb = singles.tile([128, 4], F32)
            nrm2h_sb = singles.tile([128, 4, 2], F32)
            nrm_sb = singles.tile([128, 4], F32)
            rinv_sb = singles.tile([128, 4], F32)
            lnab_sb = singles.tile([128, 4, N], F32)
            eps_sb = singles.tile([128, 1], F32)
            nc.vector.memset(eps_sb[:], 1e-30)

            nc.sync.dma_start(out=bq_sb[:], in_=bq_p.ap())
            nc.sync.dma_start(out=bk_sb[:], in_=bk_p.ap())
            bv_ap = bv_p.ap()
            nc.sync.dma_start(
                out=bvb_sb[:],
                in_=bass.AP(tensor=bv_ap.tensor, offset=bv_ap.offset,
                            ap=[[0, 128]] + list(bv_ap.ap)[1:]),
            )
            nc.vector.memset(vp_sb[:, :, :, 64:65], 1.0)

            # ---- P1: projections ----
            with tc.tile_pool(name="p1in", bufs=1) as p1in, \
                 tc.tile_pool(name="proj_ps", bufs=3, space="PSUM") as proj_ps:
                xq_sb = p1in.tile([128, 8, N], BF16, tag="xq")
                xk_sb = p1in.tile([128, 8, N], BF16, tag="xk")
                xv_sb = p1in.tile([128, 8, N], BF16, tag="xv")
                wq_sb = p1in.tile([128, 8, DQL], BF16, tag="wq")
                wk_sb = p1in.tile([128, 8, DQL], BF16, tag="wk")
                wv_sb = p1in.tile([128, 8, DQL], BF16, tag="wv")
                # Per-d-tile DMAs round-robined over four engine queues so the
                # first projection matmuls start after ~1/8 of the data lands.
                dma_engs = (nc.sync, nc.scalar, nc.gpsimd)
                qi = 0
                for p, t in ((wv_p, wv_sb), (xv_p, xv_sb),
                             (wq_p, wq_sb), (xq_p, xq_sb),
                             (wk_p, wk_sb), (xk_p, xk_sb)):
                    src = p.ap().rearrange("(t p) n -> p t n", p=128)
                    eng = dma_engs[qi % 3]
                    eng.dma_start(out=t[:, 0:2, :], in_=src[:, 0:2, :])
                    eng.dma_start(out=t[:, 2:8, :], in_=src[:, 2:8, :])
                    qi += 1
                for p, t in ((at_p, a_sb), (amt_p, am_sb)):
                    src = p.ap().rearrange("(t p) n -> t p n", p=128)
                    for dt in range(8):
                        nc.gpsimd.dma_start(out=t[:, dt, :], in_=src[dt])
                nc.gpsimd.dma_start(out=wo_sb[:], in_=wo_p.ap().rearrange("(t p) n -> p t n", p=128))

                def q_or_k_proj(w_sb, x_sb_, b_sb, o_sb, dqt):
                    for nch in range(2):
                        ps = proj_ps.tile([128, 512], F32, name=f"pp_{o_sb.tensor.name}_{dqt}_{nch}", tag="pp")
                        for dt in range(8):
                            nc.tensor.matmul(
                                ps[:],
                                w_sb[:, dt, dqt * 128:(dqt + 1) * 128],
                                x_sb_[:, dt, nch * 512:(nch + 1) * 512],
                                start=(dt == 0), stop=(dt == 7),
                            )
                        nc.scalar.activation(
                            o_sb[:, dqt, nch * 512:(nch + 1) * 512], ps[:],
                            AF.Identity, bias=b_sb[:, dqt:dqt + 1],
                        )

                def v_proj(nt):
                    ps = proj_ps.tile([128, 512], F32, name=f"pv_{nt}", tag="pp")
                    for dt in range(8):
                        nc.tensor.matmul(
                            ps[:],
                            xv_sb[:, dt, nt * 128:(nt + 1) * 128],
                            wv_sb[:, dt, :],
                            start=(dt == 0), stop=(dt == 7),
                        )
                    nc.vector.tensor_tensor(
                        out=vp_sb[:, nt, :, 0:64],
                        in0=ps[:].rearrange("p (h d) -> p h d", d=64),
                        in1=bvb_sb[:].rearrange("p (h d) -> p h d", d=64),
                        op=ALU.add,
                    )

                # Interleave Q/K/V by tile so the first attention units (which
                # need only dq-tile 0 of Q^T/K^T plus all of V') can start
                # after ~1/4 of the projection work.
                for nt in range(8):
                    v_proj(nt)
                for dqt in range(4):
                    q_or_k_proj(wq_sb, xq_sb, bq_sb, qt_sb, dqt)
                    q_or_k_proj(wk_sb, xk_sb, bk_sb, kt_sb, dqt)

            # ---- P2: attention units, with P3 interleaved per partition-tile
            # P3 (signed-sqrt + L2 normalize over sequence) uses
            # sqrt(|x|) = exp(0.5*ln(|x| + 1e-30)) so only the resident
            # Ln/Exp table set is ever needed, and the 1/||.|| factor folds
            # into the Exp bias: y = sign(x) * exp(0.5*ln|x| - 0.5*ln(nrm2)).
            with tc.tile_pool(name="s_ps", bufs=4, space="PSUM") as s_ps_pool, \
                 tc.tile_pool(name="gl_ps", bufs=2, space="PSUM") as gl_pool, \
                 tc.tile_pool(name="xp", bufs=4) as xpool, \
                 tc.tile_pool(name="ep", bufs=6) as epool, \
                 tc.tile_pool(name="up", bufs=2) as upool, \
                 tc.tile_pool(name="p3", bufs=2) as p3:
                def make_tail(u, h, qh, gl):
                    pt, po = h // 2, (h % 2) * 64
                    qs = slice(qh * 512, (qh + 1) * 512)

                    def tail():
                        lns = upool.tile([1, 1024], F32, name=f"lns_{u}", tag="lns")
                        nc.scalar.activation(lns[:], gl[64:65, :], AF.Ln)
                        r_sb = upool.tile([1, 1024], BF16, name=f"r_{u}", tag="r")
                        nc.scalar.activation(r_sb[:], lns[:], AF.Exp, scale=-1.0)
                        rbc = upool.tile([64, 1024], BF16, name=f"rbc_{u}", tag="rbc")
                        if u >= 14:
                            # Pool-engine broadcast: shorter latency chain for
                            # the last units whose tails gate P3/P4 start
                            nc.gpsimd.partition_broadcast(rbc[:], r_sb[:])
                        else:
                            nc.sync.dma_start(out=r_dram.ap()[u:u + 1, :], in_=r_sb[:])
                            rd = r_dram.ap()[u:u + 1, :]
                            nc.sync.dma_start(
                                out=rbc[:],
                                in_=bass.AP(tensor=rd.tensor, offset=rd.offset,
                                            ap=[[0, 64]] + list(rd.ap)[1:]),
                            )
                        w_sb = upool.tile([64, 1024], BF16, name=f"w_{u}", tag="w")
                        nc.vector.tensor_tensor(out=w_sb[:], in0=gl[0:64, :],
                                                in1=rbc[:], op=ALU.mult)
                        nc.vector.scalar_tensor_tensor(
                            out=tmp_sb[po:po + 64, pt, qs],
                            in0=w_sb[:, 0:512], scalar=float(rho),
                            in1=w_sb[:, 512:1024],
                            op0=ALU.mult, op1=ALU.add,
                        )
                        if h % 2 == 1:
                            # elementwise half of P3 for (pt, qh): |tmp| and its ln,
                            # plus the half-row |.| sum
                            nc.vector.tensor_reduce(
                                out=nrm2h_sb[:, pt, qh:qh + 1],
                                in_=tmp_sb[:, pt, qs],
                                axis=mybir.AxisListType.X, op=ALU.add,
                                apply_absolute_value=True,
                            )
                            abs_t = p3.tile([128, 512], BF16, name=f"abs_{u}", tag="abs")
                            nc.vector.tensor_scalar(
                                out=abs_t[:].bitcast(mybir.dt.uint16),
                                in0=tmp_sb[:, pt, qs].bitcast(mybir.dt.uint16),
                                scalar1=0x7FFF, scalar2=None, op0=ALU.bitwise_and,
                            )
                            nc.scalar.activation(lnab_sb[:, pt, qs], abs_t[:],
                                                 AF.Ln, bias=eps_sb[:])
                        if h % 2 == 1 and qh == 1:
                            # full-row P3 finish for partition-tile pt:
                            # m = -0.5*ln(max(nrm2, 1e-24));
                            # y = sign(tmp) * exp(0.5*ln|tmp| + m)
                            nc.vector.tensor_tensor(
                                out=nrm2_sb[:, pt:pt + 1],
                                in0=nrm2h_sb[:, pt, 0:1], in1=nrm2h_sb[:, pt, 1:2],
                                op=ALU.add,
                            )
                            nc.vector.tensor_scalar_max(
                                out=nrm_sb[:, pt:pt + 1], in0=nrm2_sb[:, pt:pt + 1],
                                scalar1=1e-24,
                            )
                            nc.scalar.activation(rinv_sb[:, pt:pt + 1],
                                                 nrm_sb[:, pt:pt + 1], AF.Ln)
                            nc.vector.tensor_scalar_mul(
                                out=rinv_sb[:, pt:pt + 1], in0=rinv_sb[:, pt:pt + 1],
                                scalar1=-0.5,
                            )
                            sq_t = p3.tile([128, N], BF16, name=f"sq_{u}", tag="sq")
                            nc.scalar.activation(sq_t[:], lnab_sb[:, pt, :], AF.Exp,
                                                 scale=0.5, bias=rinv_sb[:, pt:pt + 1])
                            sgn_t = p3.tile([128, N], BF16, name=f"sgn_{u}", tag="sgn")
                            nc.vector.tensor_scalar(
                                out=sgn_t[:].bitcast(mybir.dt.uint16),
                                in0=tmp_sb[:, pt, :].bitcast(mybir.dt.uint16),
                                scalar1=0x8000, scalar2=None, op0=ALU.bitwise_and,
                            )
                            nc.vector.tensor_tensor(
                                out=y_sb[:, pt, :].bitcast(mybir.dt.uint16),
                                in0=sq_t[:].bitcast(mybir.dt.uint16),
                                in1=sgn_t[:].bitcast(mybir.dt.uint16),
                                op=ALU.bitwise_or,
                            )

                    return tail

                pending_tail = None
                for u in range(16):
                    h, qh = u // 2, u % 2
                    pt, po = h // 2, (h % 2) * 64
                    qs = slice(qh * 512, (qh + 1) * 512)
                    gl = gl_pool.tile([65, 1024], F32, name=f"gl_{u}", tag="gl")
                    for kt in range(8):
                        s_ps = s_ps_pool.tile([128, 512], F32, name=f"sps_{u}_{kt}", tag="sps")
                        nc.tensor.matmul(
                            s_ps[:],
                            kt_sb[po:po + 64, pt, kt * 128:(kt + 1) * 128],
                            qt_sb[po:po + 64, pt, qs],
                            start=True, stop=True,
                        )
                        x_sb = xpool.tile([128, 512], BF16, name=f"x_{u}_{kt}", tag="x")
                        nc.scalar.activation(x_sb[:], s_ps[:], AF.Exp, scale=float(NORM))
                        e_sb = epool.tile([128, 512], BF16, name=f"e_{u}_{kt}", tag="e")
                        nc.vector.tensor_tensor(out=e_sb[:], in0=x_sb[:],
                                                in1=a_sb[:, kt, qs], op=ALU.mult)
                        em_sb = epool.tile([128, 512], BF16, name=f"em_{u}_{kt}", tag="e")
                        # Pool TTs are ~3x slower than DVE; give Pool only half
                        # the masked stream to keep it off the critical path.
                        em_eng = nc.gpsimd if kt % 2 == 0 else nc.vector
                        em_eng.tensor_tensor(out=em_sb[:], in0=x_sb[:],
                                             in1=am_sb[:, kt, qs], op=ALU.mult)
                        nc.tensor.matmul(gl[0:65, 0:512], vp_sb[:, kt, h, :], e_sb[:],
                                         start=(kt == 0), stop=(kt == 7))
                        nc.tensor.matmul(gl[0:65, 512:1024], vp_sb[:, kt, h, :], em_sb[:],
                                         start=(kt == 0), stop=(kt == 7))
                        if kt == 3 and pending_tail is not None:
                            # emit the previous unit's tail mid-stream so its
                            # 1-partition ACT work doesn't block this unit's exps
                            pending_tail()
                            pending_tail = None
                    pending_tail = make_tail(u, h, qh, gl)
                pending_tail()

            # ---- P4: output projection (partial; host sums pairs + bo) ----
            with tc.tile_pool(name="o_ps", bufs=4, space="PSUM") as o_ps_pool, \
                 tc.tile_pool(name="oc", bufs=3) as oc_pool:
                for dot in range(8):
                    for qch in range(2):
                        ps = o_ps_pool.tile([128, 512], F32, name=f"ops_{dot}_{qch}", tag="ops")
                        for dvt in range(4):
                            nc.tensor.matmul(
                                ps[:],
                                wo_sb[:, dvt, dot * 128:(dot + 1) * 128],
                                y_sb[:, dvt, qch * 512:(qch + 1) * 512],
                                start=(dvt == 0), stop=(dvt == 3),
                            )
                        ot = oc_pool.tile([128, 512], F32)
                        if (dot + qch) % 2 == 0:
                            nc.vector.tensor_copy(out=ot[:], in_=ps[:])
                        else:
                            nc.scalar.copy(out=ot[:], in_=ps[:])
                        nc.sync.dma_start(
                            out=out_p.ap()[dot * 128:(dot + 1) * 128,
                                           qch * 512:(qch + 1) * 512],
                            in_=ot[:],
                        )

    nc.finalize()
    return nc


def _get(rho: float):
    key = round(float(rho), 9)
    if key not in _CACHE:
        _CACHE[key] = _build(key)
    return _CACHE[key]


def kernel(query, key, value, adj, mask, Wq, bq, Wk, bk, Wv, bv, Wo, bo, alpha,
           _want_results=False):
    f32 = np.float32
    bf = lambda x: np.ascontiguousarray(np.asarray(x, f32)).astype(ml_dtypes.bfloat16)
    a = 1.0 / (1.0 + np.exp(-np.float64(np.asarray(alpha, f32)[0])))
    rho = float(a / (1.0 - a))
    nc = _get(rho)

    in_maps = []
    for b in range(B):
        xqT = bf(np.asarray(query[b], f32).T)
        xkT = bf(np.asarray(key[b], f32).T)
        xvT = bf(np.asarray(value[b], f32).T)
        A = np.exp(np.asarray(adj[b, 0], f32))
        Am = A * (np.asarray(mask[b, 0]) != 0)
        atT = bf(A.T)
        amtT = bf(Am.T)
        for g in range(2):
            rows = slice(g * DQL, (g + 1) * DQL)
            in_maps.append({
                "xq": xqT, "xk": xkT, "xv": xvT,
                "wq": bf(np.asarray(Wq, f32)[rows].T),
                "wk": bf(np.asarray(Wk, f32)[rows].T),
                "wv": bf(np.asarray(Wv, f32)[rows].T),
                "bq": np.ascontiguousarray(np.asarray(bq, f32)[rows].reshape(4, 128).T),
                "bk": np.ascontiguousarray(np.asarray(bk, f32)[rows].reshape(4, 128).T),
                "bv": np.ascontiguousarray(np.asarray(bv, f32)[rows].reshape(1, DQL)),
                "at": atT, "amt": amtT,
                "wo": bf(np.asarray(Wo, f32)[:, rows].T),
            })

    res = run_bass_kernel_spmd(nc, in_maps, list(range(NCORES)), trace=TRACE)
    out = np.empty((B, N, D), f32)
    bo_f = np.asarray(bo, f32)
    for b in range(B):
        out[b] = (res.results[2 * b]["out"] + res.results[2 * b + 1]["out"]).T + bo_f
    if _want_results:
        return out, res
    return out

